# revision 2
# baseline (speedup 1.0000x reference)
"""Trainium2 Bass kernel for LogicGatedSNN.

spikes = (spike_input @ ternarize(synapse_states).T >= 1.0)

v2 strategy (vs v1's bf16 hi/lo double pass):
  - Single matmul pass in float32r: the PE truncates fp32 operands to
    fp22 (11-bit mantissa) and runs at 1 cycle/row when the moving
    operand is >= 256 wide - same speed as bf16 but ~8x less error,
    well inside the 2e-2 spike-mismatch budget.  (Mixing f32r with
    bf16 operands is rejected by the compiler, so both sides are f32r;
    ternary W is exact in fp22.)
  - Roles swapped vs v1: stationary = W^T tiles [128k, 128j] (f32r),
    moving = X^T [128k, 512b] (f32r).  X^T f32 is fully SBUF-resident
    (128 KB/part) and reused across all 32 j-tiles, so W is read
    exactly once (64 MiB f32/core); each weight load amortizes over
    2x512-col moving streams.  PSUM holds out^T [128j, 512b] pairs
    (ring of 3 pairs) so j-tiles pipeline.
  - W^T built on-chip at half-j-tile granularity (16 k-tiles), ring of
    5 half-tiles, prep emitted two matmul half-windows ahead:
    DMA f32 rows -> DVE 2-op compare chunks + GpSimd combine -> one
    SBUF->SBUF xbar transpose-DMA per half -> ACT upcast bf16->f32.
    No DRAM staging round trip; DMA trigger count kept low (~1.7 us
    of sequencer time each).
  - X^T via PE transpose (f32 identity matmuls) through 2 scratch PSUM
    banks during the fill phase; X row-blocks share the W stage pool.
  - Spike threshold on DVE straight out of PSUM -> u8; device output
    is out^T [J, B]; the host transposes and widens to f32.
"""

import sys

if "/opt/trn_rl_repo" not in sys.path:
    sys.path.insert(0, "/opt/trn_rl_repo")

import numpy as np

N_CORES = 8
BATCH, IN_F, OUT_F = 8192, 4096, 4096
B_CORE = BATCH // N_CORES  # 1024

_BUILT = None


def build_bass(B, K, J, TCH=512, reps=1, out_u8=True, psum_pairs=3,
               wt_bufs=5, prime=3, tern_gpsimd=True, win_scalar=True,
               out_gpsimd=False, spk_bufs=4, x_half_first=True):
    """Per-core program: x:[B,K] f32, w:[J,K] f32 -> outT:[J,B] spikes."""
    from concourse import bacc
    import concourse.mybir as mybir
    import concourse.tile as tile

    f32, bf16, f32r = mybir.dt.float32, mybir.dt.bfloat16, mybir.dt.float32r
    u8 = mybir.dt.uint8
    alu = mybir.AluOpType
    P = 128
    BT = B // P             # b-tiles (8)
    KT = K // P             # k-tiles (32)
    JT = J // P             # j-tiles (32)
    NBC = max(1, B // 512)  # moving chunks per stationary (2)
    BC = B // NBC           # 512
    TG = min(4, BC // P)    # k-tiles per PSUM transpose group
    NH = 2 if KT >= 2 else 1
    KH = KT // NH           # k-tiles per half (16)
    KHC = KH * P            # k columns per half (2048)
    TCH = min(TCH, KHC)
    assert B % P == 0 and K % P == 0 and J % P == 0 and KT % NH == 0
    assert KHC % TCH == 0
    assert psum_pairs * NBC + 2 <= 8

    odt = u8 if out_u8 else f32

    nc = bacc.Bacc("TRN2", target_bir_lowering=False, debug=False)
    x = nc.dram_tensor("x", [B, K], f32, kind="ExternalInput")
    w = nc.dram_tensor("w", [J, K], f32, kind="ExternalInput")
    out = nc.dram_tensor("out", [J, B], odt, kind="ExternalOutput")

    with tile.TileContext(nc) as tc:
        with (
            tc.tile_pool(name="xres", bufs=1) as xres,
            tc.tile_pool(name="stage", bufs=2) as stg,
            tc.tile_pool(name="tpos", bufs=1) as tps,
            tc.tile_pool(name="tnat", bufs=2) as tnt,
            tc.tile_pool(name="wtrp", bufs=2) as wtrp,
            tc.tile_pool(name="wtp", bufs=1) as wtp,
            tc.tile_pool(name="ostage", bufs=2) as op,
            tc.tile_pool(name="psum", bufs=1, space="PSUM") as pp,
        ):
            from concourse.masks import make_identity

            ident = xres.tile([P, P], f32, name="ident")
            make_identity(nc, ident[:])

            # ---- X^T prep: whole X resident, k-major, f32 ----
            # xt[:, kt, b] = x[b, kt*128 + p]
            xt = xres.tile([P, KT, B], f32r, name="xt")

            def xprep():
                loops = (
                    [(h, b) for h in range(NH) for b in range(BT)]
                    if x_half_first else
                    [(h, b) for b in range(BT) for h in range(NH)]
                )
                for h, bsub in loops:
                    r0 = bsub * P
                    xin = stg.tile([P, KHC], f32, name="stage")
                    nc.sync.dma_start(
                        out=xin[:], in_=x[r0 : r0 + P, h * KHC : (h + 1) * KHC]
                    )
                    for g in range(KH // TG):
                        tp = pp.tile([P, TG, P], f32, name="tp", bufs=2)
                        for i in range(TG):
                            nc.tensor.transpose(
                                tp[:, i, :],
                                xin[:, (g * TG + i) * P : (g * TG + i + 1) * P],
                                ident[:],
                            )
                        kt0 = h * KH + g * TG
                        nc.scalar.copy(
                            out=xt[:, kt0 : kt0 + TG, r0 : r0 + P], in_=tp[:]
                        )

            # ---- W^T half-tile prep: 128 j rows x KHC k cols ----
            def wprep_half(jt, h):
                j0, c0 = jt * P, h * KHC
                win = stg.tile([P, KHC], f32, name="stage")
                wdma = nc.scalar if win_scalar else nc.sync
                wdma.dma_start(
                    out=win[:], in_=w[j0 : j0 + P, c0 : c0 + KHC]
                )
                t = tnt.tile([P, KHC], bf16, name="wtern")
                for c in range(KHC // TCH):
                    sl = slice(c * TCH, (c + 1) * TCH)
                    a = tps.tile([P, TCH], bf16, name="wpos")
                    nc.vector.tensor_scalar(
                        out=a[:], in0=win[:, sl], scalar1=1.0, scalar2=None,
                        op0=alu.is_gt,
                    )
                    nc.vector.tensor_scalar(
                        out=t[:, sl], in0=win[:, sl], scalar1=-1.0, scalar2=-1.0,
                        op0=alu.is_ge, op1=alu.add,
                    )
                    eng = nc.gpsimd if tern_gpsimd else nc.vector
                    eng.tensor_tensor(
                        out=t[:, sl], in0=t[:, sl], in1=a[:], op=alu.add
                    )
                wtr = wtrp.tile([P, KH, P], bf16, name="wtr")
                nc.scalar.dma_start_transpose(out=wtr[:], in_=t[:])
                wt = wtp.tile([P, KH, P], f32r, name="wt", bufs=wt_bufs)
                nc.scalar.copy(out=wt[:], in_=wtr[:])
                return wt

            for rep in range(reps):
                tasks = [(jt, h) for jt in range(JT) for h in range(NH)]
                done = {}
                emit_i = 0

                def emit_next():
                    nonlocal emit_i
                    if emit_i < len(tasks):
                        jt, h = tasks[emit_i]
                        done[(jt, h)] = wprep_half(jt, h)
                        emit_i += 1

                for _ in range(min(prime, len(tasks))):
                    emit_next()
                xprep()

                for jt in range(JT):
                    halves = [done.pop((jt, h)) for h in range(NH)]
                    psums = [
                        pp.tile([P, BC], f32, name=f"acc{bc}",
                                bufs=psum_pairs)
                        for bc in range(NBC)
                    ]
                    for k in range(KT):
                        if k % KH == 0:
                            emit_next()
                        wt = halves[k // KH]
                        stat = wt[:, k % KH, :]
                        for bc in range(NBC):
                            nc.tensor.matmul(
                                psums[bc][:],
                                stat,
                                xt[:, k, bc * BC : (bc + 1) * BC],
                                start=(k == 0),
                                stop=(k == KT - 1),
                            )
                    spk = op.tile([P, B], odt, name="spk", bufs=spk_bufs)
                    for bc in range(NBC):
                        nc.vector.tensor_scalar(
                            out=spk[:, bc * BC : (bc + 1) * BC],
                            in0=psums[bc][:],
                            scalar1=1.0, scalar2=None, op0=alu.is_ge,
                        )
                    odma = nc.gpsimd if out_gpsimd else nc.sync
                    odma.dma_start(
                        out=out[jt * P : (jt + 1) * P, :], in_=spk[:]
                    )

    nc.compile()
    return nc


def _get_built():
    global _BUILT
    if _BUILT is None:
        _BUILT = build_bass(B_CORE, IN_F, OUT_F)
    return _BUILT


def kernel(spike_input: np.ndarray, synapse_states: np.ndarray) -> np.ndarray:
    from concourse.bass_utils import run_bass_kernel_spmd

    nc = _get_built()
    xs = np.ascontiguousarray(spike_input, dtype=np.float32)
    ws = np.ascontiguousarray(synapse_states, dtype=np.float32)
    in_maps = [
        {"x": xs[c * B_CORE : (c + 1) * B_CORE], "w": ws} for c in range(N_CORES)
    ]
    res = run_bass_kernel_spmd(nc, in_maps, core_ids=list(range(N_CORES)))
    out = np.empty((BATCH, OUT_F), dtype=np.float32)
    for c in range(N_CORES):
        out[c * B_CORE : (c + 1) * B_CORE] = res.results[c]["out"].T.astype(
            np.float32
        )
    return out
